# revision 20
# baseline (speedup 1.0000x reference)
"""CRF forward (log-space scan), time-sharded across 8 TRN2 NeuronCores.

Math: alpha[t,b,j] = x[b,t,j] + logsumexp_k(alpha[t-1,b,k] + T[j,k]).
In exp space with drift normalizer c0:
    p_t = E_t * (W @ p_{t-1}),  W = exp(T),  E_t = exp(x_t - c0).

Sharding: TIME-sharded. The positive transition matrix W (entries in
[1,e]) is a Birkhoff contraction: one step shrinks projective error by
>= tanh(log_cross_ratio/4) ~ 0.463, and the diagonal emission scaling is
a projective isometry. So a core that warm-starts its 64-step segment
KW steps early from p = exp(x_s) converges to the true state DIRECTION
to ~5*0.463^KW (= 2e-5 at KW=16); the remaining per-batch SCALE offset
is constant across classes and is recovered on the host by matching the
one-step overlap between consecutive cores' segments.

Per core: all B=1024 batch rows per step, laid out as 4 chunk-groups x
32 classes on the 128 SBUF partitions (block-diagonal W) x 256 batch in
the free dim, split into 2 independent 128-column chains so TensorE /
VectorE work on one chain while the other waits on semaphores. Weights
and state bf16 (f32 PSUM accumulate), E bf16 in, p bf16 out; the host
takes log and adds back c0*t plus the per-batch stitching offsets.
"""

import numpy as np
import ml_dtypes

import concourse.bass as bass
from concourse import bacc
import concourse.mybir as mybir
from concourse import tile
from concourse.bass_utils import run_bass_kernel_spmd

B, T, C = 1024, 512, 32
NCORES = 8
SEG = T // NCORES          # 64 timesteps owned per core
KW = 12                    # warmup steps (Birkhoff washout)
N = SEG + KW               # 76 recurrence steps per core
NSLAB = 4                  # chunk-groups stacked on partitions
P = NSLAB * C              # 128 partitions
FD = B // NSLAB            # 256 batch columns per step
CHAINS = [(0, 64), (64, 96), (160, 96)]  # (col offset, width) per chain
ECH = [2, 2, 4, 4, 8, 8, 12, 12, 12, 12]   # input-chunk step counts (sum N)
OCH = [14, 14, 14, 12, 10, 4, 4, 2, 1, 1]  # output-chunk step counts (sum N)
NETRIG_TENSOR = 4          # first e-chunk DMAs triggered from idle TensorE
C0 = 4.492                 # mean per-step drift of alpha

bf16 = ml_dtypes.bfloat16

_nc_cache = None


def _build():
    global _nc_cache
    if _nc_cache is not None:
        return _nc_cache
    nc = bacc.Bacc()
    f32 = mybir.dt.float32
    bf = mybir.dt.bfloat16
    e_ext = nc.declare_dram_parameter("e", [P, N * FD], bf, isOutput=False)
    w_ext = nc.declare_dram_parameter("w", [P, P], bf, isOutput=False)
    p_ext = nc.declare_dram_parameter("p0", [P, FD], bf, isOutput=False)
    o_ext = nc.declare_dram_parameter("out", [P, N * FD], bf, isOutput=True)

    with tile.TileContext(nc) as tc:
        with (
            tc.tile_pool(name="wpool", bufs=1) as wpool,
            tc.tile_pool(name="epool", bufs=4) as epool,
            tc.tile_pool(name="opool", bufs=4) as opool,
            tc.tile_pool(name="psum", bufs=6, space="PSUM") as psum,
        ):
            # First e-chunk DMAs trigger from TensorE (dep-free, so they
            # issue the moment the engine starts); the rest from GpSimd.
            etiles = []                      # (tile, first_step, nsteps)
            s0 = 0
            wt = wpool.tile([P, P], bf, name="wt")
            nc.gpsimd.dma_start(wt[:], w_ext[:])
            p0t = wpool.tile([P, FD], bf, name="p0t")
            nc.gpsimd.dma_start(p0t[:], p_ext[:])
            for ci, ns in enumerate(ECH):
                et = epool.tile([P, ns * FD], bf, tag="e")
                nc.gpsimd.dma_start(et[:], e_ext[:, s0 * FD:(s0 + ns) * FD])
                etiles.append((et, s0, ns))
                s0 += ns

            # Prime the cross-engine semaphore paths while DMAs load: the
            # first dependent dispatch on a fresh engine pair stalls ~4us.
            dm = wpool.tile([P, 64], bf, name="dm")
            nc.vector.memset(dm[:], 0.0)
            dps = psum.tile([32, 32], f32, tag="s")
            nc.tensor.matmul(dps[:], dm[:, 0:32], dm[:, 0:32])
            nc.vector.tensor_mul(dm[0:32, 32:64], dps[:], dm[0:32, 0:32])
            nc.scalar.copy(dm[:, 0:1], dm[:, 0:1])
            ei = 0
            prev, prev_base = p0t, 0
            s0 = 0
            for ns in OCH:
                ot = opool.tile([P, ns * FD], bf, tag="o")
                for ti in range(ns):
                    j = s0 + ti                      # global step index 0..N-1
                    if j >= etiles[ei][1] + etiles[ei][2]:
                        ei += 1
                    et, e0, _ = etiles[ei]
                    base = ti * FD
                    ebase = (j - e0) * FD
                    for cofs, cw in CHAINS:
                        so = slice(base + cofs, base + cofs + cw)
                        se = slice(ebase + cofs, ebase + cofs + cw)
                        si = slice(prev_base + cofs, prev_base + cofs + cw)
                        s = psum.tile([P, cw], f32, tag="s", padded_shape=[P, 96])
                        nc.tensor.matmul(s[:], wt[:], prev[:, si])
                        nc.vector.tensor_mul(ot[:, so], s[:], et[:, se])
                    prev, prev_base = ot, base
                nc.scalar.dma_start(o_ext[:, s0 * FD:(s0 + ns) * FD], ot[:])
                s0 += ns
    nc.compile()
    _nc_cache = nc
    return nc


def _to_dev_layout(a):
    """[B, C] f32 -> [P, FD]: batch b -> (slab=b//FD)*C + class partition, b%FD col."""
    return np.ascontiguousarray(
        a.reshape(NSLAB, FD, C).transpose(0, 2, 1).reshape(P, FD))


def _starts():
    return [0 if i == 0 else SEG * i - KW for i in range(NCORES)]


def _prep_in_maps(pad_x, transition_scores, origination_scores):
    px = np.asarray(pad_x, dtype=np.float32)                       # [B, T, C]
    WT = np.exp(np.asarray(transition_scores, np.float32)).T       # lhsT[k, j]
    L = np.zeros((P, P), dtype=np.float32)
    for g in range(NSLAB):
        L[g * C:(g + 1) * C, g * C:(g + 1) * C] = WT
    Lb = L.astype(bf16)
    orig = np.asarray(origination_scores, np.float32)
    # pad one dummy step (x = c0 -> E = 1) so core 7's window stays uniform
    pxp = np.concatenate([px, np.full((B, 1, C), C0, np.float32)], axis=1)
    in_maps = []
    for i, s in enumerate(_starts()):
        ts = s + 1 + np.arange(N)
        Ei = np.exp(pxp[:, ts, :] - C0)                            # [B, N, C]
        E = Ei.reshape(NSLAB, FD, N, C).transpose(0, 3, 2, 1)      # [slab, C, N, col]
        E = E.reshape(P, N * FD)
        a0 = px[:, 0, :] + orig[None, :] if i == 0 else px[:, s, :]
        p0 = _to_dev_layout(np.exp(a0))
        in_maps.append({
            "e": np.ascontiguousarray(E).astype(bf16),
            "w": Lb,
            "p0": p0.astype(bf16),
        })
    return in_maps


def _gather(results, pad_x, origination_scores):
    px = np.asarray(pad_x, dtype=np.float64)
    orig = np.asarray(origination_scores, np.float64)
    starts = _starts()
    # device outputs -> local alphas A_i[j-1] = ln p_j + c0*j  (t = s_i + j)
    locals_ = []
    for i in range(NCORES):
        O = np.asarray(results[i]["out"])                          # bf16 [P, N*FD]
        O = (O.astype(np.float32)
              .reshape(NSLAB, C, N, FD)
              .transpose(2, 0, 3, 1)                               # [N, slab, col, C]
              .reshape(N, B, C))
        A = np.log(O).astype(np.float64)
        A += C0 * (1 + np.arange(N, dtype=np.float64))[:, None, None]
        locals_.append(A)
    # stitch per-batch scale offsets at the segment overlap points
    gammas = [np.zeros(B)]
    for i in range(1, NCORES):
        t_star = SEG * i
        jp = t_star - starts[i - 1] - 1
        jc = t_star - starts[i] - 1
        delta = np.mean(locals_[i - 1][jp] + gammas[i - 1][:, None]
                        - locals_[i][jc], axis=1)
        gammas.append(delta)
    out = np.empty((T, B, C), dtype=np.float64)
    out[0] = px[:, 0, :] + orig[None, :]
    out[1:SEG] = locals_[0][0:SEG - 1]
    for i in range(1, NCORES):
        j0 = SEG * i - starts[i] - 1
        out[SEG * i:SEG * (i + 1)] = locals_[i][j0:j0 + SEG] \
            + gammas[i][None, :, None]
    return out.astype(np.float32)


def _run(inputs, **kw):
    nc = _build()
    in_maps = _prep_in_maps(inputs["pad_x"], inputs["transition_scores"],
                            inputs["origination_scores"])
    return run_bass_kernel_spmd(nc, in_maps, list(range(NCORES)), **kw)


def _ensure_ntff_hook():
    """This image's antenv lacks axon_hooks; recreate it + register the
    ctypes NTFF hook (mirrors trn_agent_boot.trn_boot step 6)."""
    import sys
    import types
    try:
        from antenv.axon_hooks import get_axon_ntff_profile_hook  # noqa: F401
        return
    except ImportError:
        pass
    import antenv
    mod = types.ModuleType("antenv.axon_hooks")
    _h = {"hook": None}
    mod.set_axon_ntff_profile_hook = lambda h: _h.__setitem__("hook", h)
    mod.get_axon_ntff_profile_hook = lambda: _h["hook"]
    sys.modules["antenv.axon_hooks"] = mod
    antenv.axon_hooks = mod
    from trn_agent_boot.trn_boot import _ntff_profile_via_ctypes
    mod.set_axon_ntff_profile_hook(
        _ntff_profile_via_ctypes("/opt/axon/libaxon_pjrt.so"))


def run_traced(inputs, **kw):
    _ensure_ntff_hook()
    from concourse import bass_utils as bu
    bu.upload_artifacts = lambda tmpdir: "local://skipped"  # zero-egress box
    res = _run(inputs, trace=True, **kw)
    return (_gather(res.results, inputs["pad_x"], inputs["origination_scores"]),
            res.exec_time_ns)


def kernel(**inputs):
    res = _run(inputs)
    return _gather(res.results, inputs["pad_x"], inputs["origination_scores"])


# revision 21
# speedup vs baseline: 1.0016x; 1.0016x over previous
"""CRF forward (log-space scan), time-sharded across 8 TRN2 NeuronCores.

Math: alpha[t,b,j] = x[b,t,j] + logsumexp_k(alpha[t-1,b,k] + T[j,k]).
In exp space with drift normalizer c0:
    p_t = E_t * (W @ p_{t-1}),  W = exp(T),  E_t = exp(x_t - c0).

Sharding: TIME-sharded. The positive transition matrix W (entries in
[1,e]) is a Birkhoff contraction: one step shrinks projective error by
>= tanh(log_cross_ratio/4) ~ 0.463, and the diagonal emission scaling is
a projective isometry. So a core that warm-starts its 64-step segment
KW steps early from p = exp(x_s) converges to the true state DIRECTION
to ~5*0.463^KW (= 2e-5 at KW=16); the remaining per-batch SCALE offset
is constant across classes and is recovered on the host by matching the
one-step overlap between consecutive cores' segments.

Per core: all B=1024 batch rows per step, laid out as 4 chunk-groups x
32 classes on the 128 SBUF partitions (block-diagonal W) x 256 batch in
the free dim, split into 2 independent 128-column chains so TensorE /
VectorE work on one chain while the other waits on semaphores. Weights
and state bf16 (f32 PSUM accumulate), E bf16 in, p bf16 out; the host
takes log and adds back c0*t plus the per-batch stitching offsets.
"""

import numpy as np
import ml_dtypes

import concourse.bass as bass
from concourse import bacc
import concourse.mybir as mybir
from concourse import tile
from concourse.bass_utils import run_bass_kernel_spmd

B, T, C = 1024, 512, 32
NCORES = 8
SEG = T // NCORES          # 64 timesteps owned per core
KW = 12                    # warmup steps (Birkhoff washout)
N = SEG + KW               # 76 recurrence steps per core
NSLAB = 4                  # chunk-groups stacked on partitions
P = NSLAB * C              # 128 partitions
FD = B // NSLAB            # 256 batch columns per step
CHAINS = [(0, 64), (64, 96), (160, 96)]  # (col offset, width) per chain
ECH = [2, 2, 4, 4, 8, 8, 12, 12, 12, 12]   # input-chunk step counts (sum N)
OCH = [14, 14, 14, 12, 10, 4, 4, 2, 1, 1]  # output-chunk step counts (sum N)
NETRIG_TENSOR = 4          # first e-chunk DMAs triggered from idle TensorE
C0 = 4.492                 # mean per-step drift of alpha

bf16 = ml_dtypes.bfloat16

_nc_cache = None


def _build():
    global _nc_cache
    if _nc_cache is not None:
        return _nc_cache
    nc = bacc.Bacc()
    f32 = mybir.dt.float32
    bf = mybir.dt.bfloat16
    e_ext = nc.declare_dram_parameter("e", [P, N * FD], bf, isOutput=False)
    w_ext = nc.declare_dram_parameter("w", [P, P], mybir.dt.float8e4, isOutput=False)
    p_ext = nc.declare_dram_parameter("p0", [P, FD], bf, isOutput=False)
    o_ext = nc.declare_dram_parameter("out", [P, N * FD], bf, isOutput=True)

    with tile.TileContext(nc) as tc:
        with (
            tc.tile_pool(name="wpool", bufs=1) as wpool,
            tc.tile_pool(name="epool", bufs=4) as epool,
            tc.tile_pool(name="opool", bufs=4) as opool,
            tc.tile_pool(name="psum", bufs=6, space="PSUM") as psum,
        ):
            # First e-chunk DMAs trigger from TensorE (dep-free, so they
            # issue the moment the engine starts); the rest from GpSimd.
            etiles = []                      # (tile, first_step, nsteps)
            s0 = 0
            wt = wpool.tile([P, P], mybir.dt.float8e4, name="wt")
            nc.gpsimd.dma_start(wt[:], w_ext[:])
            p0t = wpool.tile([P, FD], bf, name="p0t")
            nc.gpsimd.dma_start(p0t[:], p_ext[:])
            for ci, ns in enumerate(ECH):
                et = epool.tile([P, ns * FD], bf, tag="e")
                nc.gpsimd.dma_start(et[:], e_ext[:, s0 * FD:(s0 + ns) * FD])
                etiles.append((et, s0, ns))
                s0 += ns

            # Prime the cross-engine semaphore paths while DMAs load: the
            # first dependent dispatch on a fresh engine pair stalls ~4us.
            dm = wpool.tile([P, 64], bf, name="dm")
            nc.vector.memset(dm[:], 0.0)
            dps = psum.tile([32, 32], f32, tag="s")
            nc.tensor.matmul(dps[:], dm[:, 0:32], dm[:, 0:32])
            nc.vector.tensor_mul(dm[0:32, 32:64], dps[:], dm[0:32, 0:32])
            nc.scalar.copy(dm[:, 0:1], dm[:, 0:1])
            ei = 0
            prev, prev_base = p0t, 0
            s0 = 0
            for ns in OCH:
                ot = opool.tile([P, ns * FD], bf, tag="o")
                for ti in range(ns):
                    j = s0 + ti                      # global step index 0..N-1
                    if j >= etiles[ei][1] + etiles[ei][2]:
                        ei += 1
                    et, e0, _ = etiles[ei]
                    base = ti * FD
                    ebase = (j - e0) * FD
                    for cofs, cw in CHAINS:
                        so = slice(base + cofs, base + cofs + cw)
                        se = slice(ebase + cofs, ebase + cofs + cw)
                        si = slice(prev_base + cofs, prev_base + cofs + cw)
                        s = psum.tile([P, cw], f32, tag="s", padded_shape=[P, 96])
                        nc.tensor.matmul(s[:], wt[:], prev[:, si])
                        nc.vector.tensor_mul(ot[:, so], s[:], et[:, se])
                    prev, prev_base = ot, base
                nc.scalar.dma_start(o_ext[:, s0 * FD:(s0 + ns) * FD], ot[:])
                s0 += ns
    nc.compile()
    _nc_cache = nc
    return nc


def _to_dev_layout(a):
    """[B, C] f32 -> [P, FD]: batch b -> (slab=b//FD)*C + class partition, b%FD col."""
    return np.ascontiguousarray(
        a.reshape(NSLAB, FD, C).transpose(0, 2, 1).reshape(P, FD))


def _starts():
    return [0 if i == 0 else SEG * i - KW for i in range(NCORES)]


def _prep_in_maps(pad_x, transition_scores, origination_scores):
    px = np.asarray(pad_x, dtype=np.float32)                       # [B, T, C]
    WT = np.exp(np.asarray(transition_scores, np.float32)).T       # lhsT[k, j]
    L = np.zeros((P, P), dtype=np.float32)
    for g in range(NSLAB):
        L[g * C:(g + 1) * C, g * C:(g + 1) * C] = WT
    Lb = L.astype(ml_dtypes.float8_e4m3)
    orig = np.asarray(origination_scores, np.float32)
    # pad one dummy step (x = c0 -> E = 1) so core 7's window stays uniform
    pxp = np.concatenate([px, np.full((B, 1, C), C0, np.float32)], axis=1)
    in_maps = []
    for i, s in enumerate(_starts()):
        ts = s + 1 + np.arange(N)
        Ei = np.exp(pxp[:, ts, :] - C0)                            # [B, N, C]
        E = Ei.reshape(NSLAB, FD, N, C).transpose(0, 3, 2, 1)      # [slab, C, N, col]
        E = E.reshape(P, N * FD)
        a0 = px[:, 0, :] + orig[None, :] if i == 0 else px[:, s, :]
        p0 = _to_dev_layout(np.exp(a0))
        in_maps.append({
            "e": np.ascontiguousarray(E).astype(bf16),
            "w": Lb,
            "p0": p0.astype(bf16),
        })
    return in_maps


def _gather(results, pad_x, origination_scores):
    px = np.asarray(pad_x, dtype=np.float64)
    orig = np.asarray(origination_scores, np.float64)
    starts = _starts()
    # device outputs -> local alphas A_i[j-1] = ln p_j + c0*j  (t = s_i + j)
    locals_ = []
    for i in range(NCORES):
        O = np.asarray(results[i]["out"])                          # bf16 [P, N*FD]
        O = (O.astype(np.float32)
              .reshape(NSLAB, C, N, FD)
              .transpose(2, 0, 3, 1)                               # [N, slab, col, C]
              .reshape(N, B, C))
        A = np.log(O).astype(np.float64)
        A += C0 * (1 + np.arange(N, dtype=np.float64))[:, None, None]
        locals_.append(A)
    # stitch per-batch scale offsets at the segment overlap points
    gammas = [np.zeros(B)]
    for i in range(1, NCORES):
        t_star = SEG * i
        jp = t_star - starts[i - 1] - 1
        jc = t_star - starts[i] - 1
        delta = np.mean(locals_[i - 1][jp] + gammas[i - 1][:, None]
                        - locals_[i][jc], axis=1)
        gammas.append(delta)
    out = np.empty((T, B, C), dtype=np.float64)
    out[0] = px[:, 0, :] + orig[None, :]
    out[1:SEG] = locals_[0][0:SEG - 1]
    for i in range(1, NCORES):
        j0 = SEG * i - starts[i] - 1
        out[SEG * i:SEG * (i + 1)] = locals_[i][j0:j0 + SEG] \
            + gammas[i][None, :, None]
    return out.astype(np.float32)


def _run(inputs, **kw):
    nc = _build()
    in_maps = _prep_in_maps(inputs["pad_x"], inputs["transition_scores"],
                            inputs["origination_scores"])
    return run_bass_kernel_spmd(nc, in_maps, list(range(NCORES)), **kw)


def _ensure_ntff_hook():
    """This image's antenv lacks axon_hooks; recreate it + register the
    ctypes NTFF hook (mirrors trn_agent_boot.trn_boot step 6)."""
    import sys
    import types
    try:
        from antenv.axon_hooks import get_axon_ntff_profile_hook  # noqa: F401
        return
    except ImportError:
        pass
    import antenv
    mod = types.ModuleType("antenv.axon_hooks")
    _h = {"hook": None}
    mod.set_axon_ntff_profile_hook = lambda h: _h.__setitem__("hook", h)
    mod.get_axon_ntff_profile_hook = lambda: _h["hook"]
    sys.modules["antenv.axon_hooks"] = mod
    antenv.axon_hooks = mod
    from trn_agent_boot.trn_boot import _ntff_profile_via_ctypes
    mod.set_axon_ntff_profile_hook(
        _ntff_profile_via_ctypes("/opt/axon/libaxon_pjrt.so"))


def run_traced(inputs, **kw):
    _ensure_ntff_hook()
    from concourse import bass_utils as bu
    bu.upload_artifacts = lambda tmpdir: "local://skipped"  # zero-egress box
    res = _run(inputs, trace=True, **kw)
    return (_gather(res.results, inputs["pad_x"], inputs["origination_scores"]),
            res.exec_time_ns)


def kernel(**inputs):
    res = _run(inputs)
    return _gather(res.results, inputs["pad_x"], inputs["origination_scores"])


# revision 22
# speedup vs baseline: 1.1807x; 1.1789x over previous
"""CRF forward (log-space scan), time-sharded across 8 TRN2 NeuronCores.

Math: alpha[t,b,j] = x[b,t,j] + logsumexp_k(alpha[t-1,b,k] + T[j,k]).
In exp space with drift normalizer c0:
    p_t = E_t * (W @ p_{t-1}),  W = exp(T),  E_t = exp(x_t - c0).

Sharding: TIME-sharded into 16 sub-segments of 32 steps (2 per core,
advanced side by side in one kernel step). The positive transition
matrix W (entries in [1,e]) is a Birkhoff contraction: one step shrinks
projective error by >= tanh(log_cross_ratio/4) ~ 0.463 and the diagonal
emission scaling is a projective isometry, so a sub-segment that
warm-starts KW steps early from p = exp(x_s) converges to the true
state direction to ~5*0.463^KW; the remaining per-batch scale offset is
constant across classes and is recovered on the host by matching the
one-step overlap between consecutive sub-segments.

Per kernel step: all B=1024 batch rows for BOTH sub-segments, laid out
as 4 chunk-groups x 32 classes on the 128 SBUF partitions
(block-diagonal W) x 512 free (2 subs x 256 batch), split into 4
independent 128-column chains so the serial dependency latency is
hidden behind DVE throughput. W fp8e4m3, E fp8e5m2, state/output bf16
(f32 PSUM accumulate); the host takes log and adds back c0*t plus the
per-sub stitching offsets.
"""

import numpy as np
import ml_dtypes

import concourse.bass as bass
from concourse import bacc
import concourse.mybir as mybir
from concourse import tile
from concourse.bass_utils import run_bass_kernel_spmd

B, T, C = 1024, 512, 32
NCORES = 8
NSUB = 16                  # independent time sub-segments
SSEG = T // NSUB           # 32 timesteps owned per sub-segment
KW = 12                    # warmup steps (Birkhoff washout)
NS = SSEG + KW             # 44 recurrence steps per sub-segment
NSLAB = 4                  # chunk-groups stacked on partitions
P = NSLAB * C              # 128 partitions
FD = B // NSLAB            # 256 batch columns per sub-segment step
FD2 = 2 * FD               # 512 free columns per kernel step (2 subs)
CHAINS = [(0, 128), (128, 128), (256, 128), (384, 128)]
ECH = [1, 1, 2, 2, 4, 4, 6, 8, 8, 8]   # input-chunk step counts (sum NS)
OCH = [8, 8, 8, 8, 6, 3, 2, 1]         # output-chunk step counts (sum NS)
C0 = 4.492                 # mean per-step drift of alpha

bf16 = ml_dtypes.bfloat16
f8e4 = ml_dtypes.float8_e4m3
f8e5 = ml_dtypes.float8_e5m2

_nc_cache = None


def _build():
    global _nc_cache
    if _nc_cache is not None:
        return _nc_cache
    nc = bacc.Bacc()
    f32 = mybir.dt.float32
    bf = mybir.dt.bfloat16
    e_ext = nc.declare_dram_parameter("e", [P, NS * FD2], mybir.dt.float8e5,
                                      isOutput=False)
    w_ext = nc.declare_dram_parameter("w", [P, P], mybir.dt.float8e4,
                                      isOutput=False)
    p_ext = nc.declare_dram_parameter("p0", [P, FD2], bf, isOutput=False)
    o_ext = nc.declare_dram_parameter("out", [P, NS * FD2], bf, isOutput=True)

    with tile.TileContext(nc) as tc:
        with (
            tc.tile_pool(name="wpool", bufs=1) as wpool,
            tc.tile_pool(name="epool", bufs=4) as epool,
            tc.tile_pool(name="opool", bufs=4) as opool,
            tc.tile_pool(name="psum", bufs=6, space="PSUM") as psum,
        ):
            wt = wpool.tile([P, P], mybir.dt.float8e4, name="wt")
            nc.gpsimd.dma_start(wt[:], w_ext[:])
            p0t = wpool.tile([P, FD2], bf, name="p0t")
            nc.gpsimd.dma_start(p0t[:], p_ext[:])
            etiles = []                      # (tile, first_step, nsteps)
            s0 = 0
            for ns in ECH:
                et = epool.tile([P, ns * FD2], mybir.dt.float8e5, tag="e")
                nc.gpsimd.dma_start(et[:], e_ext[:, s0 * FD2:(s0 + ns) * FD2])
                etiles.append((et, s0, ns))
                s0 += ns

            # Prime the cross-engine semaphore paths while DMAs load: the
            # first dependent dispatch on a fresh engine pair stalls ~4us.
            dm = wpool.tile([P, 64], bf, name="dm")
            nc.vector.memset(dm[:], 0.0)
            dps = psum.tile([32, 32], f32, tag="s")
            nc.tensor.matmul(dps[:], dm[:, 0:32], dm[:, 0:32])
            nc.vector.tensor_mul(dm[0:32, 32:64], dps[:], dm[0:32, 0:32])
            nc.scalar.copy(dm[:, 0:1], dm[:, 0:1])

            ei = 0
            prev, prev_base = p0t, 0
            s0 = 0
            for ns in OCH:
                ot = opool.tile([P, ns * FD2], bf, tag="o")
                for ti in range(ns):
                    j = s0 + ti                      # global step index 0..NS-1
                    if j >= etiles[ei][1] + etiles[ei][2]:
                        ei += 1
                    et, e0, _ = etiles[ei]
                    base = ti * FD2
                    ebase = (j - e0) * FD2
                    for cofs, cw in CHAINS:
                        so = slice(base + cofs, base + cofs + cw)
                        se = slice(ebase + cofs, ebase + cofs + cw)
                        si = slice(prev_base + cofs, prev_base + cofs + cw)
                        s = psum.tile([P, cw], f32, tag="s", padded_shape=[P, 128])
                        nc.tensor.matmul(s[:], wt[:], prev[:, si])
                        nc.vector.tensor_mul(ot[:, so], s[:], et[:, se])
                    prev, prev_base = ot, base
                nc.scalar.dma_start(o_ext[:, s0 * FD2:(s0 + ns) * FD2], ot[:])
                s0 += ns
    nc.compile()
    _nc_cache = nc
    return nc


def _sub_starts():
    return [0 if k == 0 else SSEG * k - KW for k in range(NSUB)]


def _prep_in_maps(pad_x, transition_scores, origination_scores):
    px = np.asarray(pad_x, dtype=np.float32)                       # [B, T, C]
    WT = np.exp(np.asarray(transition_scores, np.float32)).T       # lhsT[k, j]
    L = np.zeros((P, P), dtype=np.float32)
    for g in range(NSLAB):
        L[g * C:(g + 1) * C, g * C:(g + 1) * C] = WT
    Lb = L.astype(f8e4)
    orig = np.asarray(origination_scores, np.float32)
    # pad one dummy step (x = c0 -> E = 1) so sub 15's window stays uniform
    pxp = np.concatenate([px, np.full((B, 1, C), C0, np.float32)], axis=1)
    starts = _sub_starts()
    in_maps = []
    for i in range(NCORES):
        Es, p0s = [], []
        for k in (2 * i, 2 * i + 1):
            s = starts[k]
            ts = s + 1 + np.arange(NS)
            Es.append(np.exp(pxp[:, ts, :] - C0))                  # [B, NS, C]
            a0 = px[:, 0, :] + orig[None, :] if k == 0 else px[:, s, :]
            p0s.append(np.exp(a0))
        ES = np.stack(Es, axis=0)                                  # [2, B, NS, C]
        E = (ES.reshape(2, NSLAB, FD, NS, C)
               .transpose(1, 4, 3, 0, 2)                           # [slab,C,NS,sub,col]
               .reshape(P, NS * FD2))
        P0 = (np.stack(p0s, axis=0)                                # [2, B, C]
                .reshape(2, NSLAB, FD, C)
                .transpose(1, 3, 0, 2)                             # [slab, C, sub, col]
                .reshape(P, FD2))
        in_maps.append({
            "e": np.ascontiguousarray(E).astype(f8e5),
            "w": Lb,
            "p0": np.ascontiguousarray(P0).astype(bf16),
        })
    return in_maps


def _gather(results, pad_x, origination_scores):
    px = np.asarray(pad_x, dtype=np.float64)
    orig = np.asarray(origination_scores, np.float64)
    starts = _sub_starts()
    # device outputs -> per-sub local alphas A_k[j-1] = ln p_j + c0*j
    locals_ = []
    for i in range(NCORES):
        O = np.asarray(results[i]["out"])                          # bf16 [P, NS*FD2]
        O = (O.astype(np.float32)
              .reshape(NSLAB, C, NS, 2, FD)
              .transpose(2, 3, 0, 4, 1)                            # [NS,sub,slab,col,C]
              .reshape(NS, 2, B, C))
        A = np.log(O).astype(np.float64)
        A += C0 * (1 + np.arange(NS, dtype=np.float64))[:, None, None, None]
        locals_.append(A)
    subA = [locals_[k // 2][:, k % 2] for k in range(NSUB)]        # [NS, B, C] each
    # stitch per-batch scale offsets at the sub-segment overlap points
    gammas = [np.zeros(B)]
    for k in range(1, NSUB):
        jp = SSEG * k - starts[k - 1] - 1
        jc = KW - 1
        delta = np.mean(subA[k - 1][jp] + gammas[k - 1][:, None]
                        - subA[k][jc], axis=1)
        gammas.append(delta)
    out = np.empty((T, B, C), dtype=np.float64)
    out[0] = px[:, 0, :] + orig[None, :]
    out[1:SSEG] = subA[0][0:SSEG - 1]
    for k in range(1, NSUB):
        out[SSEG * k:SSEG * (k + 1)] = subA[k][KW - 1:KW - 1 + SSEG] \
            + gammas[k][None, :, None]
    return out.astype(np.float32)


def _run(inputs, **kw):
    nc = _build()
    in_maps = _prep_in_maps(inputs["pad_x"], inputs["transition_scores"],
                            inputs["origination_scores"])
    return run_bass_kernel_spmd(nc, in_maps, list(range(NCORES)), **kw)


def _ensure_ntff_hook():
    """This image's antenv lacks axon_hooks; recreate it + register the
    ctypes NTFF hook (mirrors trn_agent_boot.trn_boot step 6)."""
    import sys
    import types
    try:
        from antenv.axon_hooks import get_axon_ntff_profile_hook  # noqa: F401
        return
    except ImportError:
        pass
    import antenv
    mod = types.ModuleType("antenv.axon_hooks")
    _h = {"hook": None}
    mod.set_axon_ntff_profile_hook = lambda h: _h.__setitem__("hook", h)
    mod.get_axon_ntff_profile_hook = lambda: _h["hook"]
    sys.modules["antenv.axon_hooks"] = mod
    antenv.axon_hooks = mod
    from trn_agent_boot.trn_boot import _ntff_profile_via_ctypes
    mod.set_axon_ntff_profile_hook(
        _ntff_profile_via_ctypes("/opt/axon/libaxon_pjrt.so"))


def run_traced(inputs, **kw):
    _ensure_ntff_hook()
    from concourse import bass_utils as bu
    bu.upload_artifacts = lambda tmpdir: "local://skipped"  # zero-egress box
    res = _run(inputs, trace=True, **kw)
    return (_gather(res.results, inputs["pad_x"], inputs["origination_scores"]),
            res.exec_time_ns)


def kernel(**inputs):
    res = _run(inputs)
    return _gather(res.results, inputs["pad_x"], inputs["origination_scores"])


# revision 23
# speedup vs baseline: 1.2402x; 1.0504x over previous
"""CRF forward (log-space scan), time-sharded across 8 TRN2 NeuronCores.

Math: alpha[t,b,j] = x[b,t,j] + logsumexp_k(alpha[t-1,b,k] + T[j,k]).
In exp space with drift normalizer c0:
    p_t = E_t * (W @ p_{t-1}),  W = exp(T),  E_t = exp(x_t - c0).

Sharding: TIME-sharded into 16 sub-segments of 32 steps (2 per core,
advanced side by side in one kernel step). The positive transition
matrix W (entries in [1,e]) is a Birkhoff contraction: one step shrinks
projective error by >= tanh(log_cross_ratio/4) ~ 0.463 and the diagonal
emission scaling is a projective isometry, so a sub-segment that
warm-starts KW steps early from p = exp(x_s) converges to the true
state direction to ~5*0.463^KW; the remaining per-batch scale offset is
constant across classes and is recovered on the host by matching the
one-step overlap between consecutive sub-segments.

Per kernel step: all B=1024 batch rows for BOTH sub-segments, laid out
as 4 chunk-groups x 32 classes on the 128 SBUF partitions
(block-diagonal W) x 512 free (2 subs x 256 batch), split into 4
independent 128-column chains so the serial dependency latency is
hidden behind DVE throughput. W fp8e4m3, E fp8e5m2, state/output bf16
(f32 PSUM accumulate); the host takes log and adds back c0*t plus the
per-sub stitching offsets.
"""

import numpy as np
import ml_dtypes

import concourse.bass as bass
from concourse import bacc
import concourse.mybir as mybir
from concourse import tile
from concourse.bass_utils import run_bass_kernel_spmd

B, T, C = 1024, 512, 32
NCORES = 8
NSUB = 16                  # independent time sub-segments
SSEG = T // NSUB           # 32 timesteps owned per sub-segment
KW = 12                    # warmup steps (Birkhoff washout)
NS = SSEG + KW             # 44 recurrence steps per sub-segment
NSLAB = 4                  # chunk-groups stacked on partitions
P = NSLAB * C              # 128 partitions
FD = B // NSLAB            # 256 batch columns per sub-segment step
FD2 = 2 * FD               # 512 free columns per kernel step (2 subs)
CHAINS = [(0, 172), (172, 172), (344, 168)]
ECH = [1, 1, 2, 2, 4, 4, 6, 8, 8, 8]   # input-chunk step counts (sum NS)
OCH = [8, 8, 8, 8, 6, 3, 1, 1, 1]      # output-chunk step counts (sum NS)
C0 = 4.492                 # mean per-step drift of alpha

bf16 = ml_dtypes.bfloat16
f8e4 = ml_dtypes.float8_e4m3
f8e5 = ml_dtypes.float8_e5m2

_nc_cache = None


def _build():
    global _nc_cache
    if _nc_cache is not None:
        return _nc_cache
    nc = bacc.Bacc()
    f32 = mybir.dt.float32
    bf = mybir.dt.bfloat16
    e_ext = nc.declare_dram_parameter("e", [P, NS * FD2], mybir.dt.float8e5,
                                      isOutput=False)
    w_ext = nc.declare_dram_parameter("w", [P, P], mybir.dt.float8e4,
                                      isOutput=False)
    p_ext = nc.declare_dram_parameter("p0", [P, FD2], bf, isOutput=False)
    o_ext = nc.declare_dram_parameter("out", [P, NS * FD2], bf, isOutput=True)

    with tile.TileContext(nc) as tc:
        with (
            tc.tile_pool(name="wpool", bufs=1) as wpool,
            tc.tile_pool(name="epool", bufs=4) as epool,
            tc.tile_pool(name="opool", bufs=4) as opool,
            tc.tile_pool(name="psum", bufs=6, space="PSUM") as psum,
        ):
            wt = wpool.tile([P, P], mybir.dt.float8e4, name="wt")
            nc.gpsimd.dma_start(wt[:], w_ext[:])
            p0t = wpool.tile([P, FD2], bf, name="p0t")
            nc.gpsimd.dma_start(p0t[:], p_ext[:])
            etiles = []                      # (tile, first_step, nsteps)
            s0 = 0
            for ns in ECH:
                et = epool.tile([P, ns * FD2], mybir.dt.float8e5, tag="e")
                nc.gpsimd.dma_start(et[:], e_ext[:, s0 * FD2:(s0 + ns) * FD2])
                etiles.append((et, s0, ns))
                s0 += ns

            # Prime the cross-engine semaphore paths while DMAs load: the
            # first dependent dispatch on a fresh engine pair stalls ~4us.
            dm = wpool.tile([P, 64], bf, name="dm")
            nc.vector.memset(dm[:], 0.0)
            dps = psum.tile([32, 32], f32, tag="s")
            nc.tensor.matmul(dps[:], dm[:, 0:32], dm[:, 0:32])
            nc.vector.tensor_mul(dm[0:32, 32:64], dps[:], dm[0:32, 0:32])
            nc.scalar.copy(dm[:, 0:1], dm[:, 0:1])

            ei = 0
            prev, prev_base = p0t, 0
            s0 = 0
            for ns in OCH:
                ot = opool.tile([P, ns * FD2], bf, tag="o")
                for ti in range(ns):
                    j = s0 + ti                      # global step index 0..NS-1
                    if j >= etiles[ei][1] + etiles[ei][2]:
                        ei += 1
                    et, e0, _ = etiles[ei]
                    base = ti * FD2
                    ebase = (j - e0) * FD2
                    for cofs, cw in CHAINS:
                        so = slice(base + cofs, base + cofs + cw)
                        se = slice(ebase + cofs, ebase + cofs + cw)
                        si = slice(prev_base + cofs, prev_base + cofs + cw)
                        s = psum.tile([P, cw], f32, tag="s", padded_shape=[P, 512])
                        nc.tensor.matmul(s[:], wt[:], prev[:, si])
                        nc.vector.tensor_mul(ot[:, so], s[:], et[:, se])
                    prev, prev_base = ot, base
                nc.scalar.dma_start(o_ext[:, s0 * FD2:(s0 + ns) * FD2], ot[:])
                s0 += ns
    nc.compile()
    _nc_cache = nc
    return nc


def _sub_starts():
    return [0 if k == 0 else SSEG * k - KW for k in range(NSUB)]


def _prep_in_maps(pad_x, transition_scores, origination_scores):
    px = np.asarray(pad_x, dtype=np.float32)                       # [B, T, C]
    WT = np.exp(np.asarray(transition_scores, np.float32)).T       # lhsT[k, j]
    L = np.zeros((P, P), dtype=np.float32)
    for g in range(NSLAB):
        L[g * C:(g + 1) * C, g * C:(g + 1) * C] = WT
    Lb = L.astype(f8e4)
    orig = np.asarray(origination_scores, np.float32)
    # pad one dummy step (x = c0 -> E = 1) so sub 15's window stays uniform
    pxp = np.concatenate([px, np.full((B, 1, C), C0, np.float32)], axis=1)
    starts = _sub_starts()
    in_maps = []
    for i in range(NCORES):
        Es, p0s = [], []
        for k in (2 * i, 2 * i + 1):
            s = starts[k]
            ts = s + 1 + np.arange(NS)
            Es.append(np.exp(pxp[:, ts, :] - C0))                  # [B, NS, C]
            a0 = px[:, 0, :] + orig[None, :] if k == 0 else px[:, s, :]
            p0s.append(np.exp(a0))
        ES = np.stack(Es, axis=0)                                  # [2, B, NS, C]
        E = (ES.reshape(2, NSLAB, FD, NS, C)
               .transpose(1, 4, 3, 0, 2)                           # [slab,C,NS,sub,col]
               .reshape(P, NS * FD2))
        P0 = (np.stack(p0s, axis=0)                                # [2, B, C]
                .reshape(2, NSLAB, FD, C)
                .transpose(1, 3, 0, 2)                             # [slab, C, sub, col]
                .reshape(P, FD2))
        in_maps.append({
            "e": np.ascontiguousarray(E).astype(f8e5),
            "w": Lb,
            "p0": np.ascontiguousarray(P0).astype(bf16),
        })
    return in_maps


def _gather(results, pad_x, origination_scores):
    px = np.asarray(pad_x, dtype=np.float64)
    orig = np.asarray(origination_scores, np.float64)
    starts = _sub_starts()
    # device outputs -> per-sub local alphas A_k[j-1] = ln p_j + c0*j
    locals_ = []
    for i in range(NCORES):
        O = np.asarray(results[i]["out"])                          # bf16 [P, NS*FD2]
        O = (O.astype(np.float32)
              .reshape(NSLAB, C, NS, 2, FD)
              .transpose(2, 3, 0, 4, 1)                            # [NS,sub,slab,col,C]
              .reshape(NS, 2, B, C))
        A = np.log(O).astype(np.float64)
        A += C0 * (1 + np.arange(NS, dtype=np.float64))[:, None, None, None]
        locals_.append(A)
    subA = [locals_[k // 2][:, k % 2] for k in range(NSUB)]        # [NS, B, C] each
    # stitch per-batch scale offsets at the sub-segment overlap points
    gammas = [np.zeros(B)]
    for k in range(1, NSUB):
        jp = SSEG * k - starts[k - 1] - 1
        jc = KW - 1
        delta = np.mean(subA[k - 1][jp] + gammas[k - 1][:, None]
                        - subA[k][jc], axis=1)
        gammas.append(delta)
    out = np.empty((T, B, C), dtype=np.float64)
    out[0] = px[:, 0, :] + orig[None, :]
    out[1:SSEG] = subA[0][0:SSEG - 1]
    for k in range(1, NSUB):
        out[SSEG * k:SSEG * (k + 1)] = subA[k][KW - 1:KW - 1 + SSEG] \
            + gammas[k][None, :, None]
    return out.astype(np.float32)


def _run(inputs, **kw):
    nc = _build()
    in_maps = _prep_in_maps(inputs["pad_x"], inputs["transition_scores"],
                            inputs["origination_scores"])
    return run_bass_kernel_spmd(nc, in_maps, list(range(NCORES)), **kw)


def _ensure_ntff_hook():
    """This image's antenv lacks axon_hooks; recreate it + register the
    ctypes NTFF hook (mirrors trn_agent_boot.trn_boot step 6)."""
    import sys
    import types
    try:
        from antenv.axon_hooks import get_axon_ntff_profile_hook  # noqa: F401
        return
    except ImportError:
        pass
    import antenv
    mod = types.ModuleType("antenv.axon_hooks")
    _h = {"hook": None}
    mod.set_axon_ntff_profile_hook = lambda h: _h.__setitem__("hook", h)
    mod.get_axon_ntff_profile_hook = lambda: _h["hook"]
    sys.modules["antenv.axon_hooks"] = mod
    antenv.axon_hooks = mod
    from trn_agent_boot.trn_boot import _ntff_profile_via_ctypes
    mod.set_axon_ntff_profile_hook(
        _ntff_profile_via_ctypes("/opt/axon/libaxon_pjrt.so"))


def run_traced(inputs, **kw):
    _ensure_ntff_hook()
    from concourse import bass_utils as bu
    bu.upload_artifacts = lambda tmpdir: "local://skipped"  # zero-egress box
    res = _run(inputs, trace=True, **kw)
    return (_gather(res.results, inputs["pad_x"], inputs["origination_scores"]),
            res.exec_time_ns)


def kernel(**inputs):
    res = _run(inputs)
    return _gather(res.results, inputs["pad_x"], inputs["origination_scores"])


# revision 32
# speedup vs baseline: 1.2534x; 1.0107x over previous
"""CRF forward (log-space scan), time-sharded across 8 TRN2 NeuronCores.

Math: alpha[t,b,j] = x[b,t,j] + logsumexp_k(alpha[t-1,b,k] + T[j,k]).
In exp space with drift normalizer c0:
    p_t = E_t * (W @ p_{t-1}),  W = exp(T),  E_t = exp(x_t - c0).

Sharding: TIME-sharded into 16 sub-segments of 32 steps (2 per core,
advanced side by side in one kernel step). The positive transition
matrix W (entries in [1,e]) is a Birkhoff contraction: one step shrinks
projective error by >= tanh(log_cross_ratio/4) ~ 0.463 and the diagonal
emission scaling is a projective isometry, so a sub-segment that
warm-starts KW steps early from p = exp(x_s) converges to the true
state direction to ~5*0.463^KW; the remaining per-batch scale offset is
constant across classes and is recovered on the host by matching the
one-step overlap between consecutive sub-segments.

Per kernel step: all B=1024 batch rows for BOTH sub-segments, laid out
as 4 chunk-groups x 32 classes on the 128 SBUF partitions
(block-diagonal W) x 512 free (2 subs x 256 batch), split into 4
independent 128-column chains so the serial dependency latency is
hidden behind DVE throughput. W fp8e4m3, E fp8e5m2, state/output bf16
(f32 PSUM accumulate); the host takes log and adds back c0*t plus the
per-sub stitching offsets.
"""

import numpy as np
import ml_dtypes

import concourse.bass as bass
from concourse import bacc
import concourse.mybir as mybir
from concourse import tile
from concourse.bass_utils import run_bass_kernel_spmd

B, T, C = 1024, 512, 32
NCORES = 8
NSUB = 16                  # independent time sub-segments
SSEG = T // NSUB           # 32 timesteps owned per sub-segment
KW = 12                    # warmup steps (Birkhoff washout)
NS = SSEG + KW             # 44 recurrence steps per sub-segment
NSLAB = 4                  # chunk-groups stacked on partitions
P = NSLAB * C              # 128 partitions
FD = B // NSLAB            # 256 batch columns per sub-segment step
FD2 = 2 * FD               # 512 free columns per kernel step (2 subs)
CHAINS = [(0, 172), (172, 172), (344, 168)]
ECH = [1, 1, 2, 2, 4, 4, 6, 8, 8, 8]   # input-chunk step counts (sum NS)
OSKIP = KW - 1             # warmup steps whose outputs are never stored
NO = NS - OSKIP            # 33 stored steps per sub-segment
OCH = [8, 8, 8, 6, 1, 1, 1]            # output-chunk step counts (sum NO)
C0 = 4.492                 # mean per-step drift of alpha

bf16 = ml_dtypes.bfloat16
f8e4 = ml_dtypes.float8_e4m3
f8e5 = ml_dtypes.float8_e5m2

_nc_cache = None


def _build():
    global _nc_cache
    if _nc_cache is not None:
        return _nc_cache
    nc = bacc.Bacc()
    f32 = mybir.dt.float32
    bf = mybir.dt.bfloat16
    e_ext = nc.declare_dram_parameter("e", [P, NS * FD2], mybir.dt.float8e5,
                                      isOutput=False)
    w_ext = nc.declare_dram_parameter("w", [P, P], mybir.dt.float8e4,
                                      isOutput=False)
    p_ext = nc.declare_dram_parameter("p0", [P, FD2], bf, isOutput=False)
    o_ext = nc.declare_dram_parameter("out", [P, NO * FD2], bf, isOutput=True)

    with tile.TileContext(nc) as tc:
        with (
            tc.tile_pool(name="wpool", bufs=1) as wpool,
            tc.tile_pool(name="epool", bufs=4) as epool,
            tc.tile_pool(name="opool", bufs=4) as opool,
            tc.tile_pool(name="psum", bufs=6, space="PSUM") as psum,
        ):
            wt = wpool.tile([P, P], mybir.dt.float8e4, name="wt")
            nc.gpsimd.dma_start(wt[:], w_ext[:])
            p0t = wpool.tile([P, FD2], bf, name="p0t")
            nc.gpsimd.dma_start(p0t[:], p_ext[:])
            etiles = []                      # (tile, first_step, nsteps)
            s0 = 0
            for ns in ECH:
                et = epool.tile([P, ns * FD2], mybir.dt.float8e5, tag="e")
                nc.gpsimd.dma_start(et[:], e_ext[:, s0 * FD2:(s0 + ns) * FD2])
                etiles.append((et, s0, ns))
                s0 += ns

            # Prime the cross-engine semaphore paths while DMAs load: the
            # first dependent dispatch on a fresh engine pair stalls ~4us.
            dm = wpool.tile([P, 64], bf, name="dm")
            nc.vector.memset(dm[:], 0.0)
            dps = psum.tile([32, 32], f32, tag="s")
            nc.tensor.matmul(dps[:], dm[:, 0:32], dm[:, 0:32])
            nc.vector.tensor_mul(dm[0:32, 32:64], dps[:], dm[0:32, 0:32])
            nc.scalar.copy(dm[:, 0:1], dm[:, 0:1])

            warm = [wpool.tile([P, FD2], bf, name=f"warm{v}") for v in (0, 1)]
            ei = 0
            prev, prev_base = p0t, 0

            def step(j, ot, base):
                nonlocal ei, prev, prev_base
                if j >= etiles[ei][1] + etiles[ei][2]:
                    ei += 1
                et, e0, _ = etiles[ei]
                ebase = (j - e0) * FD2
                for cofs, cw in CHAINS:
                    so = slice(base + cofs, base + cofs + cw)
                    se = slice(ebase + cofs, ebase + cofs + cw)
                    si = slice(prev_base + cofs, prev_base + cofs + cw)
                    s = psum.tile([P, cw], f32, tag="s", padded_shape=[P, 512])
                    nc.tensor.matmul(s[:], wt[:], prev[:, si])
                    nc.vector.tensor_mul(ot[:, so], s[:], et[:, se])
                prev, prev_base = ot, base

            for j in range(OSKIP):               # warmup: outputs stay in SBUF
                step(j, warm[j % 2], 0)
            s0 = 0
            for ns in OCH:
                ot = opool.tile([P, ns * FD2], bf, tag="o")
                for ti in range(ns):
                    step(OSKIP + s0 + ti, ot, ti * FD2)
                nc.scalar.dma_start(o_ext[:, s0 * FD2:(s0 + ns) * FD2], ot[:])
                s0 += ns
    nc.compile()
    _nc_cache = nc
    return nc


def _sub_starts():
    return [SSEG * k - KW for k in range(NSUB)]


def _prep_in_maps(pad_x, transition_scores, origination_scores):
    px = np.asarray(pad_x, dtype=np.float32)                       # [B, T, C]
    WT = np.exp(np.asarray(transition_scores, np.float32)).T       # lhsT[k, j]
    L = np.zeros((P, P), dtype=np.float32)
    for g in range(NSLAB):
        L[g * C:(g + 1) * C, g * C:(g + 1) * C] = WT
    Lb = L.astype(f8e4)
    orig = np.asarray(origination_scores, np.float32)
    # pad one dummy step (x = c0 -> E = 1) so sub 15's window stays uniform
    pxp = np.concatenate([px, np.full((B, 1, C), C0, np.float32)], axis=1)
    starts = _sub_starts()
    # Sub 0 warm-starts from t=-KW with p=1 and dummy E=1 steps; the E at
    # t=0 is solved so the state lands exactly on exp(x_0 + orig). The
    # KW-1 dummy steps are simulated here in the device's arithmetic
    # (fp8 W blocks, bf16 state) to compute the correction divisor.
    Wq = WT.astype(f8e4).astype(np.float32)                        # [k, j]
    e_dummy = float(np.float32(np.exp(-C0)).astype(f8e5))          # quantized
    psim = np.ones((B, C), dtype=np.float32)
    for _ in range(KW - 1):
        psim = (e_dummy * (psim @ Wq)).astype(bf16).astype(np.float32)
    e_corr = np.exp(px[:, 0, :] + orig[None, :]) / (psim @ Wq)     # [B, C]
    in_maps = []
    for i in range(NCORES):
        Es, p0s = [], []
        for k in (2 * i, 2 * i + 1):
            s = starts[k]
            ts = s + 1 + np.arange(NS)
            if k == 0:
                Ei = np.full((B, NS, C), e_dummy, dtype=np.float32)
                Ei[:, KW:] = np.exp(pxp[:, ts[KW:], :] - C0)
                Ei[:, KW - 1] = e_corr
                Es.append(Ei)
                p0s.append(np.ones((B, C), dtype=np.float32))
            else:
                Es.append(np.exp(pxp[:, ts, :] - C0))              # [B, NS, C]
                p0s.append(np.exp(px[:, s, :]))
        ES = np.stack(Es, axis=0)                                  # [2, B, NS, C]
        E = (ES.reshape(2, NSLAB, FD, NS, C)
               .transpose(1, 4, 3, 0, 2)                           # [slab,C,NS,sub,col]
               .reshape(P, NS * FD2))
        P0 = (np.stack(p0s, axis=0)                                # [2, B, C]
                .reshape(2, NSLAB, FD, C)
                .transpose(1, 3, 0, 2)                             # [slab, C, sub, col]
                .reshape(P, FD2))
        in_maps.append({
            "e": np.ascontiguousarray(E).astype(f8e5),
            "w": Lb,
            "p0": np.ascontiguousarray(P0).astype(bf16),
        })
    return in_maps


def _gather(results, pad_x, origination_scores):
    px = np.asarray(pad_x, dtype=np.float64)
    orig = np.asarray(origination_scores, np.float64)
    starts = _sub_starts()
    # device outputs (stored steps OSKIP..NS-1) -> per-sub local alphas:
    # stored index o corresponds to t = SSEG*k + o - 1 for sub k; the
    # per-sub c0 convention differs by a constant, absorbed by gamma_k
    # for k >= 1 and corrected explicitly for sub 0.
    locals_ = []
    for i in range(NCORES):
        O = np.asarray(results[i]["out"])                          # bf16 [P, NO*FD2]
        O = (O.astype(np.float32)
              .reshape(NSLAB, C, NO, 2, FD)
              .transpose(2, 3, 0, 4, 1)                            # [NO,sub,slab,col,C]
              .reshape(NO, 2, B, C))
        A = np.log(O).astype(np.float64)
        A += C0 * (1 + np.arange(NO, dtype=np.float64))[:, None, None, None]
        locals_.append(A)
    subA = [locals_[k // 2][:, k % 2] for k in range(NSUB)]        # [NO, B, C] each
    subA[0] = subA[0] - C0   # sub 0: the corrected init step carries no c0
    # stitch per-batch scale offsets at the sub-segment overlap points
    gammas = [np.zeros(B)]
    for k in range(1, NSUB):
        delta = np.mean(subA[k - 1][SSEG] + gammas[k - 1][:, None]
                        - subA[k][0], axis=1)
        gammas.append(delta)
    out = np.empty((T, B, C), dtype=np.float64)
    out[0] = px[:, 0, :] + orig[None, :]
    for k in range(NSUB):
        lo = 1 if k == 0 else 0
        out[SSEG * k + lo:SSEG * (k + 1)] = subA[k][lo:SSEG] \
            + gammas[k][None, :, None]
    return out.astype(np.float32)


def _run(inputs, **kw):
    nc = _build()
    in_maps = _prep_in_maps(inputs["pad_x"], inputs["transition_scores"],
                            inputs["origination_scores"])
    return run_bass_kernel_spmd(nc, in_maps, list(range(NCORES)), **kw)


def _ensure_ntff_hook():
    """This image's antenv lacks axon_hooks; recreate it + register the
    ctypes NTFF hook (mirrors trn_agent_boot.trn_boot step 6)."""
    import sys
    import types
    try:
        from antenv.axon_hooks import get_axon_ntff_profile_hook  # noqa: F401
        return
    except ImportError:
        pass
    import antenv
    mod = types.ModuleType("antenv.axon_hooks")
    _h = {"hook": None}
    mod.set_axon_ntff_profile_hook = lambda h: _h.__setitem__("hook", h)
    mod.get_axon_ntff_profile_hook = lambda: _h["hook"]
    sys.modules["antenv.axon_hooks"] = mod
    antenv.axon_hooks = mod
    from trn_agent_boot.trn_boot import _ntff_profile_via_ctypes
    mod.set_axon_ntff_profile_hook(
        _ntff_profile_via_ctypes("/opt/axon/libaxon_pjrt.so"))


def run_traced(inputs, **kw):
    _ensure_ntff_hook()
    from concourse import bass_utils as bu
    bu.upload_artifacts = lambda tmpdir: "local://skipped"  # zero-egress box
    res = _run(inputs, trace=True, **kw)
    return (_gather(res.results, inputs["pad_x"], inputs["origination_scores"]),
            res.exec_time_ns)


def kernel(**inputs):
    res = _run(inputs)
    return _gather(res.results, inputs["pad_x"], inputs["origination_scores"])
